# revision 13
# baseline (speedup 1.0000x reference)
"""Trainium2 Bass kernel for nn_MhAttnBlock (GAT-style additive attention).

Reference computation (per batch b):
    Vproj = (V @ WV.T).reshape(k, H, 64)
    aK = K @ WK.T   (k, H)
    aQ = Q @ WQ.T   (q, H)
    w  = softmax_k(leaky_relu(aQ[q,h] + aK[k,h], 0.2))
    out[q, h*64+e] = sum_k w[q,k,h] * Vproj[k,h,e] + bias[h,e]

Key algebraic identity used on-device:
    exp(lrelu(s)) for s = aQ+aK equals max(A, B) = A + relu(B - A) with
       A = exp(aQ)*exp(aK)      (rank-1 in (q,k))
       B = exp(.2 aQ)*exp(.2 aK)
    So the score grid needs NO exp: PE builds D = B - A as a contraction-2
    matmul from tiny per-head exp vectors; a relu pass (split across ACT
    and DVE) doubles as the mandatory PSUM->SBUF move; the rank-1 A-term
    folds into the flash matmul as a C=1 accumulation.  Softmax
    denominator = ones column appended to Vproj; bias folds in via
    Vproj += bias (numerator becomes num + bias*den, so num/den = out +
    bias exactly).

This version (v2):
  - All heavy matmuls in bf16 (fp32r's LOW_HIGH replicated mode runs the
    PE at ~half rate / trips the chip power throttle when 8 cores run).
  - Flash matmul flipped: lhsT = relu-grid chunk [128k, 128q], rhs =
    Vproj head block [128k, 66] -> psO [128q, 66].  Output lands q-major
    so the entire PE-transpose epilogue of v1 disappears.
  - Front phase interleaved per-DMA-chunk (K, V, Q load order) so
    transposes/projections hide under the HBM loads.
  - Grid software-pipelined: score matmuls for tile-pair tp+1 issue
    before flash matmuls of tp, so the PE never stalls on the relu.

Sharding: data-parallel over batch B=8 across the 8 NeuronCores.
"""

import sys

for _p in ("/opt/trn_rl_repo", "/root/.axon_site/_ro/trn_rl_repo"):
    if _p not in sys.path:
        sys.path.insert(0, _p)

import numpy as np
import ml_dtypes

import concourse.bass as bass  # noqa: F401
import concourse.bacc as bacc
import concourse.mybir as mybir
import concourse.tile as tile
from concourse.masks import make_identity
from concourse.bass_utils import run_bass_kernel_spmd

F32 = mybir.dt.float32
BF16 = mybir.dt.bfloat16
FP8 = mybir.dt.float8e4
AF = mybir.ActivationFunctionType
ALU = mybir.AluOpType

B, QS, KS = 8, 1024, 1024
D = 512          # qdim = kdim = vdim
H, OD = 8, 64    # heads, head out dim
NEG = 0.2
NCORES = 8

KT = KS // 128   # 8 k-tiles
QT = QS // 128   # 8 q-tiles
DT = D // 128    # 4 d-tiles
QB = QS // 512   # 2 q-blocks of 512
NCH = 4          # dma chunks per input tensor (2 k/q-tiles each)
HB = OD + 2      # 66: [out 64 | den ones | pad]
DEN = OD         # ones/den column index within a head block


def build_kernel():
    nc = bacc.Bacc()

    # host passes Q/K/V pre-transposed (d-major) and bf16-cast; the PE
    # transposes + fp32 loads of earlier versions disappear entirely
    QTd = nc.declare_dram_parameter("QT", [D, QS], BF16, isOutput=False)
    KTd = nc.declare_dram_parameter("KT", [D, KS], BF16, isOutput=False)
    VTd = nc.declare_dram_parameter("VT", [D, KS], BF16, isOutput=False)
    # WQext/WKext: (D, 2H), col 2h = W[h,:], col 2h+1 = 0.2*W[h,:]
    WQe = nc.declare_dram_parameter("WQext", [D, 2 * H], BF16, isOutput=False)
    WKe = nc.declare_dram_parameter("WKext", [D, 2 * H], BF16, isOutput=False)
    WVT = nc.declare_dram_parameter("WVT", [D, D], BF16, isOutput=False)
    # biasext: (1, H*HB): [bias[h,:64], 0, 0] per head
    BIA = nc.declare_dram_parameter("biasext", [1, H * HB], BF16, isOutput=False)
    # sgn: (2H, 1): -1 on even partitions (negates eK1), +1 on odd
    SGN = nc.declare_dram_parameter("sgn", [2 * H, 1], F32, isOutput=False)
    OUT = nc.declare_dram_parameter("out", [QS, H * OD], F32, isOutput=True)

    with tile.TileContext(nc) as tc:
        with (
            tc.tile_pool(name="const", bufs=1) as constp,
            tc.tile_pool(name="big", bufs=1) as bigp,
            tc.tile_pool(name="stage", bufs=3) as stagep,
        ):
            # ---- tiny constants on the gpsimd DGE queue (sync stays free
            #      for the input loads; scalar queue takes the weights) ----
            identb = constp.tile([128, 128], BF16, tag="identb")
            make_identity(nc, identb[:])
            sgn_sb = constp.tile([2 * H, 1], F32, tag="sgn")
            nc.gpsimd.dma_start(sgn_sb[:], SGN[:])
            biasx = constp.tile([1, H * HB], BF16, tag="biasx")
            nc.gpsimd.dma_start(biasx[:], BIA[:])
            biasbc = constp.tile([128, H * HB], F32, tag="biasbc")
            ones1 = constp.tile([1, 128], BF16, tag="ones1")
            nc.vector.memset(ones1[:], 1.0)
            wk_sb = constp.tile([128, DT, 2 * H], BF16, tag="wk")
            nc.scalar.dma_start(
                wk_sb[:], WKe.rearrange("(dt p) j -> p dt j", p=128)
            )
            wq_sb = constp.tile([128, DT, 2 * H], BF16, tag="wq")
            nc.scalar.dma_start(
                wq_sb[:], WQe.rearrange("(dt p) j -> p dt j", p=128)
            )
            wv_sb = constp.tile([128, DT, D], BF16, tag="wv")
            nc.scalar.dma_start(
                wv_sb[:], WVT.rearrange("(dt p) e -> p dt e", p=128)
            )

            # ---- transposed input loads (K, V, Q order; 4 seq-chunks
            #      each so projections can start as data arrives) ----
            xtcm = tc.tile_pool(name="xt", bufs=1)
            xtp = xtcm.__enter__()

            def load_xt(dram, ns, tag):
                # two s-half chunks: 1KB-per-partition contiguous runs
                t_ = xtp.tile([128, DT, ns], BF16, tag=tag)
                view = dram.rearrange("(dt p) s -> p dt s", p=128)
                for c in range(2):
                    lo, hi = c * ns // 2, (c + 1) * ns // 2
                    nc.sync.dma_start(t_[:, :, lo:hi], view[:, :, lo:hi])
                return t_

            kT = load_xt(KTd, KS, "kT")
            qT = load_xt(QTd, QS, "qT")
            vT = load_xt(VTd, KS, "vT")

            # bias broadcast to 128 partitions via tiny bf16 matmul
            with tc.tile_pool(name="psbb", bufs=1, space="PSUM") as psbbp:
                psbb = psbbp.tile([128, H * HB], F32, tag="psbb")
                nc.tensor.matmul(
                    psbb[:, 0:512], lhsT=ones1[:], rhs=biasx[:, 0:512],
                    start=True, stop=True,
                )
                nc.tensor.matmul(
                    psbb[:, 512:H * HB], lhsT=ones1[:],
                    rhs=biasx[:, 512:H * HB], start=True, stop=True,
                )
                nc.vector.tensor_copy(out=biasbc[:], in_=psbb[:])

            with (
                tc.tile_pool(name="pspair", bufs=1, space="PSUM") as pspairp,
                tc.tile_pool(name="psproj", bufs=2, space="PSUM") as psprojp,
            ):
                # pair-projection psum tiles [2H, seq] fp32 (2 banks each)
                psk = pspairp.tile([2 * H, KS], F32, tag="pair")
                eK = bigp.tile([2 * H, KS], BF16, tag="ek")
                ekf = bigp.tile([2 * H, KS], F32, tag="ekf")
                eK1n = bigp.tile([128, KT, H], BF16, tag="ek1n")

                def pair_proj(ps, xt, half, w_sb):
                    for dt in range(DT):
                        nc.tensor.matmul(
                            ps[:, half * 512:(half + 1) * 512],
                            lhsT=w_sb[:, dt],
                            rhs=xt[:, dt, half * 512:(half + 1) * 512],
                            start=(dt == 0),
                            stop=(dt == DT - 1),
                        )

                def ek_finish(half):
                    sl = slice(half * 512, (half + 1) * 512)
                    nc.scalar.activation(ekf[:, sl], psk[:, sl], AF.Exp)
                    nc.vector.tensor_scalar(
                        out=eK[:, sl], in0=ekf[:, sl], scalar1=sgn_sb[:],
                        scalar2=None, op0=ALU.mult,
                    )

                def ek1n_tile(t):
                    psn_full = psprojp.tile([128, 512], F32, tag="proj",
                                            name=f"psn{t}")
                    psn = psn_full[:, 0:H]
                    for dt in range(DT):
                        nc.tensor.matmul(
                            psn,
                            lhsT=kT[:, dt, t * 128:(t + 1) * 128],
                            rhs=wk_sb[:, dt, 0:2 * H:2],
                            start=(dt == 0),
                            stop=(dt == DT - 1),
                        )
                    nc.scalar.activation(eK1n[:, t], psn, AF.Exp)

                # --- K path ---
                pair_proj(psk, kT, 0, wk_sb)
                ek_finish(0)
                for t in range(0, 4):
                    ek1n_tile(t)
                pair_proj(psk, kT, 1, wk_sb)
                ek_finish(1)
                for t in range(4, 8):
                    ek1n_tile(t)

                # stage eK pair rows at partitions 0/1 (engine APs must
                # start at partition 0/32/64/96; DMA may read anywhere);
                # gpsimd DGE queue so sync stays free for loads
                eks = bigp.tile([2, H, KS], BF16, tag="eks")
                for h in range(H):
                    nc.gpsimd.dma_start(
                        out=eks[:, h], in_=eK[2 * h:2 * h + 2, :]
                    )

                # --- Q path (before V: its projections feed the grid
                #     earliest; staged per half on the now-idle sync queue) ---
                psq = pspairp.tile([2 * H, QS], F32, tag="pair")
                eQ = bigp.tile([2 * H, QS], BF16, tag="eq")
                eqs = bigp.tile([2, H, QS], BF16, tag="eqs")
                for half in range(2):
                    sl = slice(half * 512, (half + 1) * 512)
                    pair_proj(psq, qT, half, wq_sb)
                    nc.scalar.activation(eQ[:, sl], psq[:, sl], AF.Exp)
                    for h in range(H):
                        nc.sync.dma_start(
                            out=eqs[:, h, sl], in_=eQ[2 * h:2 * h + 2, sl]
                        )

                # --- V path: Vproj' per tile: [Vproj_h + bias_h | 1 | 0] ---
                vp_sb = bigp.tile([128, KT, H * HB], BF16, tag="vp")
                nc.vector.memset(
                    vp_sb[:].rearrange("p t (h e) -> p t h e", h=H)[
                        :, :, :, DEN:DEN + 1
                    ],
                    1.0,
                )
                nc.vector.memset(
                    vp_sb[:].rearrange("p t (h e) -> p t h e", h=H)[
                        :, :, :, DEN + 1:HB
                    ],
                    0.0,
                )

                def vp_tile(t):
                    psv = psprojp.tile([128, 512], F32, tag="proj",
                                       name=f"psv{t}")
                    for dt in range(DT):
                        nc.tensor.matmul(
                            psv[:],
                            lhsT=vT[:, dt, t * 128:(t + 1) * 128],
                            rhs=wv_sb[:, dt],
                            start=(dt == 0),
                            stop=(dt == DT - 1),
                        )
                    nc.vector.tensor_tensor(
                        out=vp_sb[:, t].rearrange("p (h e) -> p h e", h=H)[
                            :, :, 0:OD
                        ],
                        in0=psv[:].rearrange("p (h e) -> p h e", h=H),
                        in1=biasbc[:].rearrange("p (h e) -> p h e", h=H)[
                            :, :, 0:OD
                        ],
                        op=ALU.add,
                    )

                vp8 = bigp.tile([128, KT, H * HB], FP8, tag="vp8")
                for t in range(KT):
                    vp_tile(t)
                    nc.scalar.copy(out=vp8[:, t], in_=vp_sb[:, t])

                # --- cv: cv1'[h] = sum_k exp(aK_h)[k] * Vp'[k, block h] ---
                # one [8, 264] matmul pair per k-tile; head h's block sits at
                # cols h*66 of the concatenated [8, 528] result (diag blocks)
                with tc.tile_pool(name="cvp", bufs=1, space="PSUM") as cvpp:
                    # [H, 2, 512] so each half's [8, 264] matmul output sits
                    # at a PSUM bank boundary (offsets 0 and 2048 bytes)
                    cvps = cvpp.tile([H, 2, 512], F32, tag="cvps")
                    for t in range(KT):
                        for hh in range(2):
                            nc.tensor.matmul(
                                cvps[:, hh, 0:4 * HB],
                                lhsT=eK1n[:, t],
                                rhs=vp_sb[:, t, hh * 4 * HB:(hh + 1) * 4 * HB],
                                start=(t == 0),
                                stop=(t == KT - 1),
                            )
                    cvf = bigp.tile([H, 2 * 4 * HB], BF16, tag="cvf")
                    for hh in range(2):
                        nc.vector.tensor_copy(
                            out=cvf[:, hh * 4 * HB:(hh + 1) * 4 * HB],
                            in_=cvps[:, hh, 0:4 * HB],
                        )
                # gather diagonal blocks to partition 0: cv_sb[0, h*66+j]
                cv_sb = constp.tile([1, H * HB], BF16, tag="cv")
                for h in range(H):
                    nc.gpsimd.dma_start(
                        out=cv_sb[:, h * HB:(h + 1) * HB],
                        in_=cvf[h:h + 1, h * HB:(h + 1) * HB],
                    )

            xtcm.__exit__(None, None, None)

            # ---- main grid: score D = B - A (bf16), relu -> fp8,
            #      flash via fp8 DoubleRow (2 k-tiles per pass, r moving),
            #      psO [66, 512] then bf16 PE-transpose epilogue ----
            outv = OUT.rearrange("(t p) e -> p t e", p=128)
            with (
                tc.tile_pool(name="psd", bufs=2, space="PSUM") as psdp,
                tc.tile_pool(name="pso", bufs=2, space="PSUM") as psop,
                tc.tile_pool(name="ps2", bufs=2, space="PSUM") as ps2p,
                tc.tile_pool(name="rpool", bufs=9) as rp,
                tc.tile_pool(name="outf", bufs=4) as outfp,
            ):
                NIT = QB * H  # 16 iterations, j -> (qb, h)
                TP = KT // 2
                psO = [None] * NIT
                rsb = [None] * NIT

                def emit_scores(j):
                    qb, h = divmod(j, H)
                    qs = qb * 512
                    rsb[j] = []
                    for tp in range(TP):
                        ps = psdp.tile([128, 1024], F32, tag="psd",
                                       name=f"psD{j}_{tp}")
                        for i in range(2):
                            t = tp * 2 + i
                            nc.tensor.matmul(
                                ps[:, i * 512:(i + 1) * 512],
                                lhsT=eks[:, h, t * 128:(t + 1) * 128],
                                rhs=eqs[:, h, qs:qs + 512],
                                start=True, stop=True,
                            )
                        r = rp.tile([128, 2, 512], FP8, tag="r",
                                    name=f"r{j}_{tp}")
                        rsb[j].append(r)
                        # relu split: ACT 576 cols, DVE 448 cols (fp8 out)
                        rf = r[:].rearrange("p i q -> p (i q)")
                        nc.scalar.activation(rf[:, 0:576], ps[:, 0:576],
                                             AF.Relu)
                        nc.vector.tensor_scalar(
                            out=rf[:, 576:1024], in0=ps[:, 576:1024],
                            scalar1=0.0, scalar2=None, op0=ALU.max,
                        )

                def emit_flashepi(j):
                    qb, h = divmod(j, H)
                    qs = qb * 512
                    pso_t = psop.tile([66, 512], F32, tag="pso",
                                      name=f"psO{j}")
                    psO[j] = pso_t
                    # rank-1 A-term opens the chain: psO[f, q] += cv[f]*eq1[q]
                    nc.tensor.matmul(
                        pso_t[:],
                        lhsT=cv_sb[0:1, h * HB:(h + 1) * HB],
                        rhs=eqs[0:1, h, qs:qs + 512],
                        start=True, stop=False,
                    )
                    for tp in range(TP):
                        nc.tensor.matmul(
                            pso_t[:],
                            lhsT=vp8[:, 2 * tp:2 * tp + 2,
                                     h * HB:(h + 1) * HB],
                            rhs=rsb[j][tp][:],
                            start=False, stop=(tp == TP - 1),
                            perf_mode=mybir.MatmulPerfMode.DoubleRow,
                        )
                    rsb[j] = None
                    # epilogue: copy to bf16, transpose to q-major, divide
                    o_sb = outfp.tile([66, 512], BF16, tag="osb",
                                      name=f"osb{j}")
                    nc.vector.tensor_copy(out=o_sb[:], in_=pso_t[:])
                    ps2 = ps2p.tile([128, 4 * HB], BF16, tag="ps2",
                                    name=f"ps2_{j}")
                    for c in range(4):
                        nc.tensor.transpose(
                            ps2[:, c * HB:(c + 1) * HB],
                            o_sb[:, c * 128:(c + 1) * 128],
                            identb[0:HB, 0:HB],
                        )
                    rden = stagep.tile([128, 4], F32, tag="rden",
                                       name=f"rden{j}")
                    nc.vector.reciprocal(
                        rden[:],
                        ps2[:].rearrange("p (c e) -> p c e", c=4)[
                            :, :, DEN:DEN + 1
                        ],
                    )
                    oF = outfp.tile([128, 4 * OD], F32, tag="of",
                                    name=f"oF{j}")
                    for c in range(4):
                        nc.vector.tensor_scalar(
                            out=oF[:, c * OD:(c + 1) * OD],
                            in0=ps2[:, c * HB:c * HB + OD],
                            scalar1=rden[:, c:c + 1],
                            scalar2=None,
                            op0=ALU.mult,
                        )
                    nc.sync.dma_start(
                        out=outv[:, qb * 4:(qb + 1) * 4, h * OD:(h + 1) * OD],
                        in_=oF[:].rearrange("p (c e) -> p c e", c=4),
                    )
                    psO[j] = None

                emit_scores(0)
                for j in range(NIT):
                    if j + 1 < NIT:
                        emit_scores(j + 1)
                    emit_flashepi(j)
    nc.compile()
    return nc


_NC_CACHE = {}


def _get_nc():
    if "nc" not in _NC_CACHE:
        _NC_CACHE["nc"] = build_kernel()
    return _NC_CACHE["nc"]


def make_inmaps(Q, K, V, WQ, WK, WV, bias):
    Q = np.asarray(Q, np.float32)
    K = np.asarray(K, np.float32)
    V = np.asarray(V, np.float32)
    WQ = np.asarray(WQ, np.float32)
    WK = np.asarray(WK, np.float32)
    WV = np.asarray(WV, np.float32)
    bias = np.asarray(bias, np.float32)

    def ext(W):  # (H, D) -> (D, 2H), col 2h = W[h], col 2h+1 = .2*W[h]
        e = np.empty((D, 2 * H), np.float32)
        e[:, 0::2] = W.T
        e[:, 1::2] = NEG * W.T
        return e.astype(ml_dtypes.bfloat16)

    wqe = ext(WQ)
    wke = ext(WK)
    wvt = np.ascontiguousarray(WV.T).astype(ml_dtypes.bfloat16)
    biasext = np.zeros((1, H * HB), np.float32)
    biasext.reshape(H, HB)[:, 0:OD] = bias
    biasext = biasext.astype(ml_dtypes.bfloat16)
    sgn = np.tile(np.array([[-1.0], [1.0]], np.float32), (H, 1))

    # pre-transpose Q/K/V to d-major bf16 (batched transpose, then cast)
    QTb = np.ascontiguousarray(Q.transpose(0, 2, 1)).astype(ml_dtypes.bfloat16)
    KTb = np.ascontiguousarray(K.transpose(0, 2, 1)).astype(ml_dtypes.bfloat16)
    VTb = np.ascontiguousarray(V.transpose(0, 2, 1)).astype(ml_dtypes.bfloat16)

    in_maps = []
    for b in range(NCORES):
        in_maps.append({
            "QT": QTb[b],
            "KT": KTb[b],
            "VT": VTb[b],
            "WQext": wqe,
            "WKext": wke,
            "WVT": wvt,
            "biasext": biasext,
            "sgn": sgn,
        })
    return in_maps


def kernel(Q, K, V, WQ, WK, WV, bias):
    nc = _get_nc()
    in_maps = make_inmaps(Q, K, V, WQ, WK, WV, bias)
    res = run_bass_kernel_spmd(nc, in_maps, list(range(NCORES)))
    out = np.stack([res.results[b]["out"] for b in range(NCORES)], axis=0)
    return out


# revision 14
# speedup vs baseline: 1.0085x; 1.0085x over previous
"""Trainium2 Bass kernel for nn_MhAttnBlock (GAT-style additive attention).

Reference computation (per batch b):
    Vproj = (V @ WV.T).reshape(k, H, 64)
    aK = K @ WK.T   (k, H)
    aQ = Q @ WQ.T   (q, H)
    w  = softmax_k(leaky_relu(aQ[q,h] + aK[k,h], 0.2))
    out[q, h*64+e] = sum_k w[q,k,h] * Vproj[k,h,e] + bias[h,e]

Key algebraic identity used on-device:
    exp(lrelu(s)) for s = aQ+aK equals max(A, B) = A + relu(B - A) with
       A = exp(aQ)*exp(aK)      (rank-1 in (q,k))
       B = exp(.2 aQ)*exp(.2 aK)
    So the score grid needs NO exp: PE builds D = B - A as a contraction-2
    matmul from tiny per-head exp vectors; a relu pass (split across ACT
    and DVE) doubles as the mandatory PSUM->SBUF move; the rank-1 A-term
    folds into the flash matmul as a C=1 accumulation.  Softmax
    denominator = ones column appended to Vproj; bias folds in via
    Vproj += bias (numerator becomes num + bias*den, so num/den = out +
    bias exactly).

This version (v2):
  - All heavy matmuls in bf16 (fp32r's LOW_HIGH replicated mode runs the
    PE at ~half rate / trips the chip power throttle when 8 cores run).
  - Flash matmul flipped: lhsT = relu-grid chunk [128k, 128q], rhs =
    Vproj head block [128k, 66] -> psO [128q, 66].  Output lands q-major
    so the entire PE-transpose epilogue of v1 disappears.
  - Front phase interleaved per-DMA-chunk (K, V, Q load order) so
    transposes/projections hide under the HBM loads.
  - Grid software-pipelined: score matmuls for tile-pair tp+1 issue
    before flash matmuls of tp, so the PE never stalls on the relu.

Sharding: data-parallel over batch B=8 across the 8 NeuronCores.
"""

import sys

for _p in ("/opt/trn_rl_repo", "/root/.axon_site/_ro/trn_rl_repo"):
    if _p not in sys.path:
        sys.path.insert(0, _p)

import numpy as np
import ml_dtypes

import concourse.bass as bass  # noqa: F401
import concourse.bacc as bacc
import concourse.mybir as mybir
import concourse.tile as tile
from concourse.bass_utils import run_bass_kernel_spmd

F32 = mybir.dt.float32
BF16 = mybir.dt.bfloat16
AF = mybir.ActivationFunctionType
ALU = mybir.AluOpType

B, QS, KS = 8, 1024, 1024
D = 512          # qdim = kdim = vdim
H, OD = 8, 64    # heads, head out dim
NEG = 0.2
NCORES = 8

KT = KS // 128   # 8 k-tiles
QT = QS // 128   # 8 q-tiles
DT = D // 128    # 4 d-tiles
QB = QS // 512   # 2 q-blocks of 512
NCH = 4          # dma chunks per input tensor (2 k/q-tiles each)
HB = OD + 2      # 66: [out 64 | den ones | pad]
DEN = OD         # ones/den column index within a head block


def build_kernel():
    nc = bacc.Bacc()

    # host passes Q/K/V pre-transposed (d-major) and bf16-cast; the PE
    # transposes + fp32 loads of earlier versions disappear entirely
    QTd = nc.declare_dram_parameter("QT", [D, QS], BF16, isOutput=False)
    KTd = nc.declare_dram_parameter("KT", [D, KS], BF16, isOutput=False)
    VTd = nc.declare_dram_parameter("VT", [D, KS], BF16, isOutput=False)
    # WQext/WKext: (D, 2H), col 2h = W[h,:], col 2h+1 = 0.2*W[h,:]
    WQe = nc.declare_dram_parameter("WQext", [D, 2 * H], BF16, isOutput=False)
    WKe = nc.declare_dram_parameter("WKext", [D, 2 * H], BF16, isOutput=False)
    WVT = nc.declare_dram_parameter("WVT", [D, D], BF16, isOutput=False)
    # biasext: (1, H*HB): [bias[h,:64], 0, 0] per head
    BIA = nc.declare_dram_parameter("biasext", [1, H * HB], BF16, isOutput=False)
    # sgn: (2H, 1): -1 on even partitions (negates eK1), +1 on odd
    SGN = nc.declare_dram_parameter("sgn", [2 * H, 1], F32, isOutput=False)
    OUT = nc.declare_dram_parameter("out", [QS, H * OD], F32, isOutput=True)

    with tile.TileContext(nc) as tc:
        with (
            tc.tile_pool(name="const", bufs=1) as constp,
            tc.tile_pool(name="big", bufs=1) as bigp,
            tc.tile_pool(name="stage", bufs=3) as stagep,
        ):
            # ---- tiny constants on the gpsimd DGE queue (sync stays free
            #      for the input loads; scalar queue takes the weights) ----
            sgn_sb = constp.tile([2 * H, 1], F32, tag="sgn")
            nc.gpsimd.dma_start(sgn_sb[:], SGN[:])
            biasx = constp.tile([1, H * HB], BF16, tag="biasx")
            nc.gpsimd.dma_start(biasx[:], BIA[:])
            biasbc = constp.tile([128, H * HB], F32, tag="biasbc")
            ones1 = constp.tile([1, 128], BF16, tag="ones1")
            nc.vector.memset(ones1[:], 1.0)
            wk_sb = constp.tile([128, DT, 2 * H], BF16, tag="wk")
            nc.scalar.dma_start(
                wk_sb[:], WKe.rearrange("(dt p) j -> p dt j", p=128)
            )
            wq_sb = constp.tile([128, DT, 2 * H], BF16, tag="wq")
            nc.scalar.dma_start(
                wq_sb[:], WQe.rearrange("(dt p) j -> p dt j", p=128)
            )
            wv_sb = constp.tile([128, DT, D], BF16, tag="wv")
            nc.scalar.dma_start(
                wv_sb[:], WVT.rearrange("(dt p) e -> p dt e", p=128)
            )

            # ---- transposed input loads (K, V, Q order; 4 seq-chunks
            #      each so projections can start as data arrives) ----
            xtcm = tc.tile_pool(name="xt", bufs=1)
            xtp = xtcm.__enter__()

            def load_xt(dram, ns, tag):
                # two s-half chunks: 1KB-per-partition contiguous runs
                t_ = xtp.tile([128, DT, ns], BF16, tag=tag)
                view = dram.rearrange("(dt p) s -> p dt s", p=128)
                for c in range(2):
                    lo, hi = c * ns // 2, (c + 1) * ns // 2
                    nc.sync.dma_start(t_[:, :, lo:hi], view[:, :, lo:hi])
                return t_

            kT = load_xt(KTd, KS, "kT")
            qT = load_xt(QTd, QS, "qT")
            vT = load_xt(VTd, KS, "vT")

            # bias broadcast to 128 partitions via tiny bf16 matmul
            with tc.tile_pool(name="psbb", bufs=1, space="PSUM") as psbbp:
                psbb = psbbp.tile([128, H * HB], F32, tag="psbb")
                nc.tensor.matmul(
                    psbb[:, 0:512], lhsT=ones1[:], rhs=biasx[:, 0:512],
                    start=True, stop=True,
                )
                nc.tensor.matmul(
                    psbb[:, 512:H * HB], lhsT=ones1[:],
                    rhs=biasx[:, 512:H * HB], start=True, stop=True,
                )
                nc.vector.tensor_copy(out=biasbc[:], in_=psbb[:])

            with (
                tc.tile_pool(name="pspair", bufs=1, space="PSUM") as pspairp,
                tc.tile_pool(name="psproj", bufs=2, space="PSUM") as psprojp,
            ):
                # pair-projection psum tiles [2H, seq] fp32 (2 banks each)
                psk = pspairp.tile([2 * H, KS], F32, tag="pair")
                eK = bigp.tile([2 * H, KS], BF16, tag="ek")
                ekf = bigp.tile([2 * H, KS], F32, tag="ekf")
                eK1n = bigp.tile([128, KT, H], BF16, tag="ek1n")

                def pair_proj(ps, xt, half, w_sb):
                    for dt in range(DT):
                        nc.tensor.matmul(
                            ps[:, half * 512:(half + 1) * 512],
                            lhsT=w_sb[:, dt],
                            rhs=xt[:, dt, half * 512:(half + 1) * 512],
                            start=(dt == 0),
                            stop=(dt == DT - 1),
                        )

                def ek_finish(half):
                    sl = slice(half * 512, (half + 1) * 512)
                    nc.scalar.activation(ekf[:, sl], psk[:, sl], AF.Exp)
                    nc.vector.tensor_scalar(
                        out=eK[:, sl], in0=ekf[:, sl], scalar1=sgn_sb[:],
                        scalar2=None, op0=ALU.mult,
                    )

                def ek1n_tile(t):
                    psn_full = psprojp.tile([128, 512], F32, tag="proj",
                                            name=f"psn{t}")
                    psn = psn_full[:, 0:H]
                    for dt in range(DT):
                        nc.tensor.matmul(
                            psn,
                            lhsT=kT[:, dt, t * 128:(t + 1) * 128],
                            rhs=wk_sb[:, dt, 0:2 * H:2],
                            start=(dt == 0),
                            stop=(dt == DT - 1),
                        )
                    nc.scalar.activation(eK1n[:, t], psn, AF.Exp)

                # --- K path ---
                pair_proj(psk, kT, 0, wk_sb)
                ek_finish(0)
                for t in range(0, 4):
                    ek1n_tile(t)
                pair_proj(psk, kT, 1, wk_sb)
                ek_finish(1)
                for t in range(4, 8):
                    ek1n_tile(t)

                # stage eK pair rows at partitions 0/1 (engine APs must
                # start at partition 0/32/64/96; DMA may read anywhere);
                # gpsimd DGE queue so sync stays free for loads
                eks = bigp.tile([2, H, KS], BF16, tag="eks")
                for h in range(H):
                    nc.gpsimd.dma_start(
                        out=eks[:, h], in_=eK[2 * h:2 * h + 2, :]
                    )

                # --- Q path (before V: its projections feed the grid
                #     earliest; staged per half on the now-idle sync queue) ---
                psq = pspairp.tile([2 * H, QS], F32, tag="pair")
                eQ = bigp.tile([2 * H, QS], BF16, tag="eq")
                eqs = bigp.tile([2, H, QS], BF16, tag="eqs")
                for half in range(2):
                    sl = slice(half * 512, (half + 1) * 512)
                    pair_proj(psq, qT, half, wq_sb)
                    nc.scalar.activation(eQ[:, sl], psq[:, sl], AF.Exp)
                    for h in range(H):
                        nc.sync.dma_start(
                            out=eqs[:, h, sl], in_=eQ[2 * h:2 * h + 2, sl]
                        )

                # --- V path: Vproj' per tile: [Vproj_h + bias_h | 1 | 0] ---
                vp_sb = bigp.tile([128, KT, H * HB], BF16, tag="vp")
                nc.vector.memset(
                    vp_sb[:].rearrange("p t (h e) -> p t h e", h=H)[
                        :, :, :, DEN:DEN + 1
                    ],
                    1.0,
                )
                nc.vector.memset(
                    vp_sb[:].rearrange("p t (h e) -> p t h e", h=H)[
                        :, :, :, DEN + 1:HB
                    ],
                    0.0,
                )

                def vp_tile(t):
                    psv = psprojp.tile([128, 512], F32, tag="proj",
                                       name=f"psv{t}")
                    for dt in range(DT):
                        nc.tensor.matmul(
                            psv[:],
                            lhsT=vT[:, dt, t * 128:(t + 1) * 128],
                            rhs=wv_sb[:, dt],
                            start=(dt == 0),
                            stop=(dt == DT - 1),
                        )
                    nc.vector.tensor_tensor(
                        out=vp_sb[:, t].rearrange("p (h e) -> p h e", h=H)[
                            :, :, 0:OD
                        ],
                        in0=psv[:].rearrange("p (h e) -> p h e", h=H),
                        in1=biasbc[:].rearrange("p (h e) -> p h e", h=H)[
                            :, :, 0:OD
                        ],
                        op=ALU.add,
                    )

                for t in range(KT):
                    vp_tile(t)

                # --- cv: cv1'[h] = sum_k exp(aK_h)[k] * Vp'[k, block h] ---
                # one [8, 264] matmul pair per k-tile; head h's block sits at
                # cols h*66 of the concatenated [8, 528] result (diag blocks)
                with tc.tile_pool(name="cvp", bufs=1, space="PSUM") as cvpp:
                    # [H, 2, 512] so each half's [8, 264] matmul output sits
                    # at a PSUM bank boundary (offsets 0 and 2048 bytes)
                    cvps = cvpp.tile([H, 2, 512], F32, tag="cvps")
                    for t in range(KT):
                        for hh in range(2):
                            nc.tensor.matmul(
                                cvps[:, hh, 0:4 * HB],
                                lhsT=eK1n[:, t],
                                rhs=vp_sb[:, t, hh * 4 * HB:(hh + 1) * 4 * HB],
                                start=(t == 0),
                                stop=(t == KT - 1),
                            )
                    cvf = bigp.tile([H, 2 * 4 * HB], BF16, tag="cvf")
                    for hh in range(2):
                        nc.vector.tensor_copy(
                            out=cvf[:, hh * 4 * HB:(hh + 1) * 4 * HB],
                            in_=cvps[:, hh, 0:4 * HB],
                        )
                # gather diagonal blocks to partition 0: cv_sb[0, h*66+j]
                cv_sb = constp.tile([1, H * HB], BF16, tag="cv")
                for h in range(H):
                    nc.gpsimd.dma_start(
                        out=cv_sb[:, h * HB:(h + 1) * HB],
                        in_=cvf[h:h + 1, h * HB:(h + 1) * HB],
                    )

            xtcm.__exit__(None, None, None)

            # ---- main grid: score D = B - A, relu, flipped flash ----
            # PSUM accumulation chains within one bank must be strictly
            # sequential (no two open groups in a bank region), so each
            # (qb,h) runs its 4 q-chunk chains back to back; the software
            # pipeline instead runs one full (qb,h) iteration ahead on the
            # score side.
            outv = OUT.rearrange("(t p) e -> p t e", p=128)
            with (
                tc.tile_pool(name="psd", bufs=3, space="PSUM") as psdp,
                tc.tile_pool(name="pso", bufs=2, space="PSUM") as psop,
                tc.tile_pool(name="rpool", bufs=9) as rp,
                tc.tile_pool(name="outf", bufs=4) as outfp,
            ):
                NIT = QB * H  # 16 iterations, j -> (qb, h)
                TP = KT // 2
                psO = [None] * NIT
                rsb = [None] * NIT

                def emit_scores(j):
                    qb, h = divmod(j, H)
                    qs = qb * 512
                    rsb[j] = []
                    for tp in range(TP):
                        ps = psdp.tile([128, 1024], F32, tag="psd",
                                       name=f"psD{j}_{tp}")
                        for i in range(2):
                            t = tp * 2 + i
                            nc.tensor.matmul(
                                ps[:, i * 512:(i + 1) * 512],
                                lhsT=eks[:, h, t * 128:(t + 1) * 128],
                                rhs=eqs[:, h, qs:qs + 512],
                                start=True, stop=True,
                            )
                        r = rp.tile([128, 1024], BF16, tag="r",
                                    name=f"r{j}_{tp}")
                        rsb[j].append(r)
                        # relu split: ACT 576 cols, DVE 448 cols
                        nc.scalar.activation(r[:, 0:576], ps[:, 0:576],
                                             AF.Relu)
                        nc.vector.tensor_scalar(
                            out=r[:, 576:1024], in0=ps[:, 576:1024],
                            scalar1=0.0, scalar2=None, op0=ALU.max,
                        )

                def emit_flashepi(j):
                    qb, h = divmod(j, H)
                    qs = qb * 512
                    pso_t = psop.tile([128, 4 * HB], F32, tag="pso",
                                      name=f"psO{j}")
                    psO[j] = pso_t
                    for c in range(4):
                        # rank-1 A-term opens chunk c's accumulation chain
                        nc.tensor.matmul(
                            pso_t[:, c * HB:(c + 1) * HB],
                            lhsT=eqs[0:1, h, qs + c * 128:qs + (c + 1) * 128],
                            rhs=cv_sb[0:1, h * HB:(h + 1) * HB],
                            start=True, stop=False,
                        )
                        for tp in range(TP):
                            r = rsb[j][tp]
                            for i in range(2):
                                t = tp * 2 + i
                                nc.tensor.matmul(
                                    pso_t[:, c * HB:(c + 1) * HB],
                                    lhsT=r[:, i * 512 + c * 128:
                                           i * 512 + (c + 1) * 128],
                                    rhs=vp_sb[:, t, h * HB:(h + 1) * HB],
                                    start=False, stop=(t == KT - 1),
                                )
                    rsb[j] = None
                    # epilogue: reciprocal of den column, scale, store
                    rden = stagep.tile([128, 4], F32, tag="rden",
                                       name=f"rden{j}")
                    nc.vector.reciprocal(
                        rden[:],
                        pso_t[:].rearrange("p (c e) -> p c e", c=4)[
                            :, :, DEN:DEN + 1
                        ],
                    )
                    oF = outfp.tile([128, 4 * OD], F32, tag="of",
                                    name=f"oF{j}")
                    for c in range(4):
                        nc.vector.tensor_scalar(
                            out=oF[:, c * OD:(c + 1) * OD],
                            in0=pso_t[:, c * HB:c * HB + OD],
                            scalar1=rden[:, c:c + 1],
                            scalar2=None,
                            op0=ALU.mult,
                        )
                    nc.sync.dma_start(
                        out=outv[:, qb * 4:(qb + 1) * 4, h * OD:(h + 1) * OD],
                        in_=oF[:].rearrange("p (c e) -> p c e", c=4),
                    )
                    psO[j] = None

                emit_scores(0)
                for j in range(NIT):
                    if j + 1 < NIT:
                        emit_scores(j + 1)
                    emit_flashepi(j)
    nc.compile()
    return nc


_NC_CACHE = {}


def _get_nc():
    if "nc" not in _NC_CACHE:
        _NC_CACHE["nc"] = build_kernel()
    return _NC_CACHE["nc"]


def make_inmaps(Q, K, V, WQ, WK, WV, bias):
    Q = np.asarray(Q, np.float32)
    K = np.asarray(K, np.float32)
    V = np.asarray(V, np.float32)
    WQ = np.asarray(WQ, np.float32)
    WK = np.asarray(WK, np.float32)
    WV = np.asarray(WV, np.float32)
    bias = np.asarray(bias, np.float32)

    def ext(W):  # (H, D) -> (D, 2H), col 2h = W[h], col 2h+1 = .2*W[h]
        e = np.empty((D, 2 * H), np.float32)
        e[:, 0::2] = W.T
        e[:, 1::2] = NEG * W.T
        return e.astype(ml_dtypes.bfloat16)

    wqe = ext(WQ)
    wke = ext(WK)
    wvt = np.ascontiguousarray(WV.T).astype(ml_dtypes.bfloat16)
    biasext = np.zeros((1, H * HB), np.float32)
    biasext.reshape(H, HB)[:, 0:OD] = bias
    biasext = biasext.astype(ml_dtypes.bfloat16)
    sgn = np.tile(np.array([[-1.0], [1.0]], np.float32), (H, 1))

    # pre-transpose Q/K/V to d-major bf16 (batched transpose, then cast)
    QTb = np.ascontiguousarray(Q.transpose(0, 2, 1)).astype(ml_dtypes.bfloat16)
    KTb = np.ascontiguousarray(K.transpose(0, 2, 1)).astype(ml_dtypes.bfloat16)
    VTb = np.ascontiguousarray(V.transpose(0, 2, 1)).astype(ml_dtypes.bfloat16)

    in_maps = []
    for b in range(NCORES):
        in_maps.append({
            "QT": QTb[b],
            "KT": KTb[b],
            "VT": VTb[b],
            "WQext": wqe,
            "WKext": wke,
            "WVT": wvt,
            "biasext": biasext,
            "sgn": sgn,
        })
    return in_maps


def kernel(Q, K, V, WQ, WK, WV, bias):
    nc = _get_nc()
    in_maps = make_inmaps(Q, K, V, WQ, WK, WV, bias)
    res = run_bass_kernel_spmd(nc, in_maps, list(range(NCORES)))
    out = np.stack([res.results[b]["out"] for b in range(NCORES)], axis=0)
    return out


# revision 15
# speedup vs baseline: 1.0581x; 1.0492x over previous
"""Trainium2 Bass kernel for nn_MhAttnBlock (GAT-style additive attention).

Reference computation (per batch b):
    Vproj = (V @ WV.T).reshape(k, H, 64)
    aK = K @ WK.T   (k, H)
    aQ = Q @ WQ.T   (q, H)
    w  = softmax_k(leaky_relu(aQ[q,h] + aK[k,h], 0.2))
    out[q, h*64+e] = sum_k w[q,k,h] * Vproj[k,h,e] + bias[h,e]

Key algebraic identity used on-device:
    exp(lrelu(s)) for s = aQ+aK equals max(A, B) = A + relu(B - A) with
       A = exp(aQ)*exp(aK)      (rank-1 in (q,k))
       B = exp(.2 aQ)*exp(.2 aK)
    So the score grid needs NO exp: PE builds D = B - A as a contraction-2
    matmul from tiny per-head exp vectors; a relu pass (split across ACT
    and DVE) doubles as the mandatory PSUM->SBUF move; the rank-1 A-term
    folds into the flash matmul as a C=1 accumulation.  Softmax
    denominator = ones column appended to Vproj; bias folds in via
    Vproj += bias (numerator becomes num + bias*den, so num/den = out +
    bias exactly).

This version (v2):
  - All heavy matmuls in bf16 (fp32r's LOW_HIGH replicated mode runs the
    PE at ~half rate / trips the chip power throttle when 8 cores run).
  - Flash matmul flipped: lhsT = relu-grid chunk [128k, 128q], rhs =
    Vproj head block [128k, 66] -> psO [128q, 66].  Output lands q-major
    so the entire PE-transpose epilogue of v1 disappears.
  - Front phase interleaved per-DMA-chunk (K, V, Q load order) so
    transposes/projections hide under the HBM loads.
  - Grid software-pipelined: score matmuls for tile-pair tp+1 issue
    before flash matmuls of tp, so the PE never stalls on the relu.

Sharding: data-parallel over batch B=8 across the 8 NeuronCores.
"""

import sys

for _p in ("/opt/trn_rl_repo", "/root/.axon_site/_ro/trn_rl_repo"):
    if _p not in sys.path:
        sys.path.insert(0, _p)

import numpy as np
import ml_dtypes

import concourse.bass as bass  # noqa: F401
import concourse.bacc as bacc
import concourse.mybir as mybir
import concourse.tile as tile
from concourse.masks import make_identity
from concourse.bass_utils import run_bass_kernel_spmd

F32 = mybir.dt.float32
BF16 = mybir.dt.bfloat16
AF = mybir.ActivationFunctionType
ALU = mybir.AluOpType

B, QS, KS = 8, 1024, 1024
D = 512          # qdim = kdim = vdim
H, OD = 8, 64    # heads, head out dim
NEG = 0.2
NCORES = 8

KT = KS // 128   # 8 k-tiles
QT = QS // 128   # 8 q-tiles
DT = D // 128    # 4 d-tiles
QB = QS // 512   # 2 q-blocks of 512
NCH = 4          # dma chunks per input tensor (2 k/q-tiles each)
HB = OD + 2      # 66: [out 64 | den ones | pad]
DEN = OD         # ones/den column index within a head block


def build_kernel():
    nc = bacc.Bacc()

    # host passes Q/K/V pre-transposed (d-major) and bf16-cast; the PE
    # transposes + fp32 loads of earlier versions disappear entirely
    QTd = nc.declare_dram_parameter("QT", [D, QS], BF16, isOutput=False)
    KTd = nc.declare_dram_parameter("KT", [D, KS], BF16, isOutput=False)
    VTd = nc.declare_dram_parameter("VT", [D, KS], BF16, isOutput=False)
    # WQext/WKext: (D, 2H), col 2h = W[h,:], col 2h+1 = 0.2*W[h,:]
    WQe = nc.declare_dram_parameter("WQext", [D, 2 * H], BF16, isOutput=False)
    WKe = nc.declare_dram_parameter("WKext", [D, 2 * H], BF16, isOutput=False)
    WVT = nc.declare_dram_parameter("WVT", [D, D], BF16, isOutput=False)
    # biasext: (1, H*HB): [bias[h,:64], 0, 0] per head
    BIA = nc.declare_dram_parameter("biasext", [1, H * HB], BF16, isOutput=False)
    # sgn: (2H, 1): -1 on even partitions (negates eK1), +1 on odd
    SGN = nc.declare_dram_parameter("sgn", [2 * H, 1], F32, isOutput=False)
    OUT = nc.declare_dram_parameter("out", [QS, H * OD], F32, isOutput=True)

    with tile.TileContext(nc) as tc:
        with (
            tc.tile_pool(name="const", bufs=1) as constp,
            tc.tile_pool(name="big", bufs=1) as bigp,
            tc.tile_pool(name="stage", bufs=3) as stagep,
        ):
            # ---- tiny constants on the gpsimd DGE queue (sync stays free
            #      for the input loads; scalar queue takes the weights) ----
            identb = constp.tile([128, 128], BF16, tag="identb")
            make_identity(nc, identb[:])
            sgn_sb = constp.tile([2 * H, 1], F32, tag="sgn")
            nc.gpsimd.dma_start(sgn_sb[:], SGN[:])
            biasx = constp.tile([1, H * HB], BF16, tag="biasx")
            nc.gpsimd.dma_start(biasx[:], BIA[:])
            biasbc = constp.tile([128, H * HB], F32, tag="biasbc")
            ones1 = constp.tile([1, 128], BF16, tag="ones1")
            nc.vector.memset(ones1[:], 1.0)
            wk_sb = constp.tile([128, DT, 2 * H], BF16, tag="wk")
            nc.scalar.dma_start(
                wk_sb[:], WKe.rearrange("(dt p) j -> p dt j", p=128)
            )
            wq_sb = constp.tile([128, DT, 2 * H], BF16, tag="wq")
            nc.scalar.dma_start(
                wq_sb[:], WQe.rearrange("(dt p) j -> p dt j", p=128)
            )
            wv_sb = constp.tile([128, DT, D], BF16, tag="wv")
            nc.scalar.dma_start(
                wv_sb[:], WVT.rearrange("(dt p) e -> p dt e", p=128)
            )

            # ---- transposed input loads (K, V, Q order; 4 seq-chunks
            #      each so projections can start as data arrives) ----
            xtcm = tc.tile_pool(name="xt", bufs=1)
            xtp = xtcm.__enter__()

            def load_xt(dram, ns, tag):
                # two s-half chunks: 1KB-per-partition contiguous runs
                t_ = xtp.tile([128, DT, ns], BF16, tag=tag)
                view = dram.rearrange("(dt p) s -> p dt s", p=128)
                for c in range(2):
                    lo, hi = c * ns // 2, (c + 1) * ns // 2
                    nc.sync.dma_start(t_[:, :, lo:hi], view[:, :, lo:hi])
                return t_

            kT = load_xt(KTd, KS, "kT")
            qT = load_xt(QTd, QS, "qT")
            vT = load_xt(VTd, KS, "vT")

            # bias broadcast to 128 partitions via tiny bf16 matmul
            with tc.tile_pool(name="psbb", bufs=1, space="PSUM") as psbbp:
                psbb = psbbp.tile([128, H * HB], F32, tag="psbb")
                nc.tensor.matmul(
                    psbb[:, 0:512], lhsT=ones1[:], rhs=biasx[:, 0:512],
                    start=True, stop=True,
                )
                nc.tensor.matmul(
                    psbb[:, 512:H * HB], lhsT=ones1[:],
                    rhs=biasx[:, 512:H * HB], start=True, stop=True,
                )
                nc.vector.tensor_copy(out=biasbc[:], in_=psbb[:])

            with (
                tc.tile_pool(name="pspair", bufs=1, space="PSUM") as pspairp,
                tc.tile_pool(name="psproj", bufs=2, space="PSUM") as psprojp,
                tc.tile_pool(name="psnt", bufs=2, space="PSUM") as psntp,
            ):
                # pair-projection psum tiles [2H, seq] fp32 (2 banks each)
                psk = pspairp.tile([2 * H, KS], F32, tag="pair")
                eK = bigp.tile([2 * H, KS], BF16, tag="ek")
                ekf = bigp.tile([2 * H, KS], F32, tag="ekf")
                eK1n = bigp.tile([128, KT, H], BF16, tag="ek1n")

                def pair_proj(ps, xt, half, w_sb):
                    for dt in range(DT):
                        nc.tensor.matmul(
                            ps[:, half * 512:(half + 1) * 512],
                            lhsT=w_sb[:, dt],
                            rhs=xt[:, dt, half * 512:(half + 1) * 512],
                            start=(dt == 0),
                            stop=(dt == DT - 1),
                        )

                def ek_finish(half):
                    sl = slice(half * 512, (half + 1) * 512)
                    nc.scalar.activation(ekf[:, sl], psk[:, sl], AF.Exp)
                    nc.vector.tensor_scalar(
                        out=eK[:, sl], in0=ekf[:, sl], scalar1=sgn_sb[:],
                        scalar2=None, op0=ALU.mult,
                    )

                e1b = bigp.tile([H, KS], BF16, tag="e1b")

                def ek1n_tiles(trange):
                    for t in trange:
                        psn = psntp.tile([128, H], BF16, tag="psnt",
                                         name=f"psnt{t}")
                        nc.tensor.transpose(
                            psn[:],
                            e1b[:, t * 128:(t + 1) * 128],
                            identb[0:H, 0:H],
                        )
                        nc.vector.tensor_copy(out=eK1n[:, t], in_=psn[:])

                # --- K path ---
                pair_proj(psk, kT, 0, wk_sb)
                ek_finish(0)
                nc.vector.tensor_copy(out=e1b[:, 0:512], in_=ekf[0:H, 0:512])
                ek1n_tiles(range(0, 4))
                pair_proj(psk, kT, 1, wk_sb)
                ek_finish(1)
                nc.vector.tensor_copy(out=e1b[:, 512:KS], in_=ekf[0:H, 512:KS])
                ek1n_tiles(range(4, 8))

                # stage eK pair rows at partitions 0/1 (engine APs must
                # start at partition 0/32/64/96; DMA may read anywhere);
                # gpsimd DGE queue so sync stays free for loads.  WKext is
                # half-split, so head h's rows are h (-eK1) and h+8 (eK2).
                eks = bigp.tile([2, H, KS], BF16, tag="eks")
                for h in range(H):
                    nc.gpsimd.dma_start(
                        out=eks[0:1, h], in_=eK[h:h + 1, :]
                    )
                    nc.gpsimd.dma_start(
                        out=eks[1:2, h], in_=eK[h + 8:h + 9, :]
                    )

                # --- Q path (before V: its projections feed the grid
                #     earliest; staged per half on the now-idle sync queue) ---
                psq = pspairp.tile([2 * H, QS], F32, tag="pair")
                eQ = bigp.tile([2 * H, QS], BF16, tag="eq")
                eqs = bigp.tile([2, H, QS], BF16, tag="eqs")
                for half in range(2):
                    sl = slice(half * 512, (half + 1) * 512)
                    pair_proj(psq, qT, half, wq_sb)
                    nc.scalar.activation(eQ[:, sl], psq[:, sl], AF.Exp)
                    for h in range(H):
                        nc.sync.dma_start(
                            out=eqs[:, h, sl], in_=eQ[2 * h:2 * h + 2, sl]
                        )

                # --- V path: Vproj' per tile: [Vproj_h + bias_h | 1 | 0] ---
                vp_sb = bigp.tile([128, KT, H * HB], BF16, tag="vp")
                nc.vector.memset(
                    vp_sb[:].rearrange("p t (h e) -> p t h e", h=H)[
                        :, :, :, DEN:DEN + 1
                    ],
                    1.0,
                )
                nc.vector.memset(
                    vp_sb[:].rearrange("p t (h e) -> p t h e", h=H)[
                        :, :, :, DEN + 1:HB
                    ],
                    0.0,
                )

                def vp_tile(t):
                    psv = psprojp.tile([128, 512], F32, tag="proj",
                                       name=f"psv{t}")
                    for dt in range(DT):
                        nc.tensor.matmul(
                            psv[:],
                            lhsT=vT[:, dt, t * 128:(t + 1) * 128],
                            rhs=wv_sb[:, dt],
                            start=(dt == 0),
                            stop=(dt == DT - 1),
                        )
                    nc.vector.tensor_tensor(
                        out=vp_sb[:, t].rearrange("p (h e) -> p h e", h=H)[
                            :, :, 0:OD
                        ],
                        in0=psv[:].rearrange("p (h e) -> p h e", h=H),
                        in1=biasbc[:].rearrange("p (h e) -> p h e", h=H)[
                            :, :, 0:OD
                        ],
                        op=ALU.add,
                    )

                for t in range(KT):
                    vp_tile(t)

                # --- cv: cv1'[h] = sum_k exp(aK_h)[k] * Vp'[k, block h] ---
                # one [8, 264] matmul pair per k-tile; head h's block sits at
                # cols h*66 of the concatenated [8, 528] result (diag blocks)
                with tc.tile_pool(name="cvp", bufs=1, space="PSUM") as cvpp:
                    # [H, 2, 512] so each half's [8, 264] matmul output sits
                    # at a PSUM bank boundary (offsets 0 and 2048 bytes)
                    cvps = cvpp.tile([H, 2, 512], F32, tag="cvps")
                    for t in range(KT):
                        for hh in range(2):
                            nc.tensor.matmul(
                                cvps[:, hh, 0:4 * HB],
                                lhsT=eK1n[:, t],
                                rhs=vp_sb[:, t, hh * 4 * HB:(hh + 1) * 4 * HB],
                                start=(t == 0),
                                stop=(t == KT - 1),
                            )
                    cvf = bigp.tile([H, 2 * 4 * HB], BF16, tag="cvf")
                    for hh in range(2):
                        nc.vector.tensor_copy(
                            out=cvf[:, hh * 4 * HB:(hh + 1) * 4 * HB],
                            in_=cvps[:, hh, 0:4 * HB],
                        )
                # gather diagonal blocks to partition 0: cv_sb[0, h*66+j]
                cv_sb = constp.tile([1, H * HB], BF16, tag="cv")
                for h in range(H):
                    nc.gpsimd.dma_start(
                        out=cv_sb[:, h * HB:(h + 1) * HB],
                        in_=cvf[h:h + 1, h * HB:(h + 1) * HB],
                    )

            xtcm.__exit__(None, None, None)

            # ---- main grid: score D = B - A, relu, flipped flash ----
            # PSUM accumulation chains within one bank must be strictly
            # sequential (no two open groups in a bank region), so each
            # (qb,h) runs its 4 q-chunk chains back to back; the software
            # pipeline instead runs one full (qb,h) iteration ahead on the
            # score side.
            outv = OUT.rearrange("(t p) e -> p t e", p=128)
            with (
                tc.tile_pool(name="psd", bufs=3, space="PSUM") as psdp,
                tc.tile_pool(name="pso", bufs=2, space="PSUM") as psop,
                tc.tile_pool(name="rpool", bufs=9) as rp,
                tc.tile_pool(name="outf", bufs=4) as outfp,
            ):
                NIT = QB * H  # 16 iterations, j -> (qb, h)
                TP = KT // 2
                psO = [None] * NIT
                rsb = [None] * NIT

                def emit_scores(j):
                    qb, h = divmod(j, H)
                    qs = qb * 512
                    rsb[j] = []
                    for tp in range(TP):
                        ps = psdp.tile([128, 1024], F32, tag="psd",
                                       name=f"psD{j}_{tp}")
                        for i in range(2):
                            t = tp * 2 + i
                            nc.tensor.matmul(
                                ps[:, i * 512:(i + 1) * 512],
                                lhsT=eks[:, h, t * 128:(t + 1) * 128],
                                rhs=eqs[:, h, qs:qs + 512],
                                start=True, stop=True,
                            )
                        r = rp.tile([128, 1024], BF16, tag="r",
                                    name=f"r{j}_{tp}")
                        rsb[j].append(r)
                        # relu split: ACT 576 cols, DVE 448 cols
                        nc.scalar.activation(r[:, 0:576], ps[:, 0:576],
                                             AF.Relu)
                        nc.vector.tensor_scalar(
                            out=r[:, 576:1024], in0=ps[:, 576:1024],
                            scalar1=0.0, scalar2=None, op0=ALU.max,
                        )

                def emit_flashepi(j):
                    qb, h = divmod(j, H)
                    qs = qb * 512
                    pso_t = psop.tile([128, 4 * HB], F32, tag="pso",
                                      name=f"psO{j}")
                    psO[j] = pso_t
                    for c in range(4):
                        # rank-1 A-term opens chunk c's accumulation chain
                        nc.tensor.matmul(
                            pso_t[:, c * HB:(c + 1) * HB],
                            lhsT=eqs[0:1, h, qs + c * 128:qs + (c + 1) * 128],
                            rhs=cv_sb[0:1, h * HB:(h + 1) * HB],
                            start=True, stop=False,
                        )
                        for tp in range(TP):
                            r = rsb[j][tp]
                            for i in range(2):
                                t = tp * 2 + i
                                nc.tensor.matmul(
                                    pso_t[:, c * HB:(c + 1) * HB],
                                    lhsT=r[:, i * 512 + c * 128:
                                           i * 512 + (c + 1) * 128],
                                    rhs=vp_sb[:, t, h * HB:(h + 1) * HB],
                                    start=False, stop=(t == KT - 1),
                                )
                    rsb[j] = None
                    # epilogue: reciprocal of den column, scale, store
                    rden = stagep.tile([128, 4], F32, tag="rden",
                                       name=f"rden{j}")
                    nc.vector.reciprocal(
                        rden[:],
                        pso_t[:].rearrange("p (c e) -> p c e", c=4)[
                            :, :, DEN:DEN + 1
                        ],
                    )
                    oF = outfp.tile([128, 4 * OD], F32, tag="of",
                                    name=f"oF{j}")
                    for c in range(4):
                        nc.vector.tensor_scalar(
                            out=oF[:, c * OD:(c + 1) * OD],
                            in0=pso_t[:, c * HB:c * HB + OD],
                            scalar1=rden[:, c:c + 1],
                            scalar2=None,
                            op0=ALU.mult,
                        )
                    nc.sync.dma_start(
                        out=outv[:, qb * 4:(qb + 1) * 4, h * OD:(h + 1) * OD],
                        in_=oF[:].rearrange("p (c e) -> p c e", c=4),
                    )
                    psO[j] = None

                emit_scores(0)
                for j in range(NIT):
                    if j + 1 < NIT:
                        emit_scores(j + 1)
                    emit_flashepi(j)
    nc.compile()
    return nc


_NC_CACHE = {}


def _get_nc():
    if "nc" not in _NC_CACHE:
        _NC_CACHE["nc"] = build_kernel()
    return _NC_CACHE["nc"]


def make_inmaps(Q, K, V, WQ, WK, WV, bias):
    Q = np.asarray(Q, np.float32)
    K = np.asarray(K, np.float32)
    V = np.asarray(V, np.float32)
    WQ = np.asarray(WQ, np.float32)
    WK = np.asarray(WK, np.float32)
    WV = np.asarray(WV, np.float32)
    bias = np.asarray(bias, np.float32)

    def ext(W):  # (H, D) -> (D, 2H), col 2h = W[h], col 2h+1 = .2*W[h]
        e = np.empty((D, 2 * H), np.float32)
        e[:, 0::2] = W.T
        e[:, 1::2] = NEG * W.T
        return e.astype(ml_dtypes.bfloat16)

    wqe = ext(WQ)
    # WKext half-split: cols 0..7 = W rows, cols 8..15 = 0.2*W rows
    wke = np.concatenate([WK.T, NEG * WK.T], axis=1).astype(ml_dtypes.bfloat16)
    wvt = np.ascontiguousarray(WV.T).astype(ml_dtypes.bfloat16)
    biasext = np.zeros((1, H * HB), np.float32)
    biasext.reshape(H, HB)[:, 0:OD] = bias
    biasext = biasext.astype(ml_dtypes.bfloat16)
    # eK sign: -1 on rows 0..7 (negates eK1), +1 on rows 8..15 (eK2)
    sgn = np.concatenate([-np.ones((H, 1)), np.ones((H, 1))]).astype(np.float32)

    # pre-transpose Q/K/V to d-major bf16 (batched transpose, then cast)
    QTb = np.ascontiguousarray(Q.transpose(0, 2, 1)).astype(ml_dtypes.bfloat16)
    KTb = np.ascontiguousarray(K.transpose(0, 2, 1)).astype(ml_dtypes.bfloat16)
    VTb = np.ascontiguousarray(V.transpose(0, 2, 1)).astype(ml_dtypes.bfloat16)

    in_maps = []
    for b in range(NCORES):
        in_maps.append({
            "QT": QTb[b],
            "KT": KTb[b],
            "VT": VTb[b],
            "WQext": wqe,
            "WKext": wke,
            "WVT": wvt,
            "biasext": biasext,
            "sgn": sgn,
        })
    return in_maps


def kernel(Q, K, V, WQ, WK, WV, bias):
    nc = _get_nc()
    in_maps = make_inmaps(Q, K, V, WQ, WK, WV, bias)
    res = run_bass_kernel_spmd(nc, in_maps, list(range(NCORES)))
    out = np.stack([res.results[b]["out"] for b in range(NCORES)], axis=0)
    return out
